# revision 14
# baseline (speedup 1.0000x reference)
"""Multi-head causal attention (B=4, C=2048, E=1024, H=16, D=64) on 8 trn2 cores.

Sharding: core i = (batch b=i//2, head-group g=i%2).  Each core computes its
batch's attention for 8 heads (512 features) and a partial output projection;
the host sums the two partials per batch (W_o split row-wise).

Per-core pipeline (ACT-exp is the pacing engine; tensor work is interleaved
into its gaps):
  - Q/K projections in fp8e4 DoubleRow (x and W_q/W_k host-quantized to fp8;
    W_q/W_k host-scaled by 16 to avoid fp8 subnormals and column-permuted so
    the psum partition layout is the packed [4 heads x 32 feats] x 2 slabs
    needed by the score matmuls).  4 DR matmuls per 512-token psum.
  - Scores S^T per head: fp8 DoubleRow, lhsT/rhs = 32-partition slices of the
    packed K^T/Q^T tiles (contraction 64 = 32 partitions x 2 slabs).
    Diagonal q-chunks are column-trimmed (only q >= k-block start computed).
  - exp on ACT with scale 1/2048 (scores carry the 16x16 weight scaling),
    output bf16 wt; triangular mask multiply on DVE for diagonal blocks only.
  - hid = [V|1s]^T @ wt in bf16, accumulated over k-blocks diag-first so the
    trimmed column ranges have a full-width first (start) and last (stop)
    matmul.  Even head of a pair uses [V|ones] (hid rows 0:64, rowsum rows
    64:128); odd head uses [ones|V] so its normalize multiply can write hidT
    rows 64:128 directly (DVE requires aligned SBUF operands; one PSUM input
    with a shifted SBUF operand is allowed).
  - 1/rowsum via exp(-ln(rs)) on ACT (reciprocal's table doesn't coexist with
    exp's; ln/exp share one set), then DVE mult psum-hid x recip -> hidT SBUF.
  - out = hidT^T @ W_o in bf16, interleaved per 4-qb chunk as soon as the
    previous q-chunk's hidT completes; psum -> f32 staging -> DRAM.
Scheduling: qc-outer sweep ordered so quad0's first two q-chunks run while
quad1's QK projections (and streamed V projections) fill the tensor gaps.
"""

import numpy as np

import concourse.bass as bass
import concourse.mybir as mybir
import concourse.tile as tile
from concourse.vector_clock import ScopedClock

B, C, E = 4, 2048, 1024
H, D = 16, 64
N_CORES = 8
GF = 512          # features per head-group (8 heads x 64)
QC = 512          # q-chunk width
KB = 128          # k-block width
NQC = C // QC     # 4
NKB = C // KB     # 16
NE = E // 128     # 8 contraction tiles over E
F32 = mybir.dt.float32
BF16 = mybir.dt.bfloat16
FP8 = mybir.dt.float8e4
DR = mybir.MatmulPerfMode.DoubleRow
SC = 16.0                   # host scale on W_q/W_k (fp8 subnormal avoidance)
EXPSCALE = 0.125 / (SC * SC)  # 1/sqrt(D) / (16*16)

_CACHED_NC = None


class PatchedTC(tile.TileContext):
    """This walrus build caps sync waits per instruction (1 for CTRL, ~2 for
    compute ISA structs).  Hoist excess waits onto same-engine NOPs emitted
    just before the instruction (engine streams execute in order, so the
    semantics are identical), and split the end-of-kernel drain's waits
    across single-wait drain instructions."""

    WAIT_CAP = 1

    def _commit_instruction(self, inst, lazy_reg_writes=True):
        si = getattr(inst, "sync_info", None)
        if (
            si is not None
            and len(si.on_wait) > self.WAIT_CAP
            and getattr(inst, "engine", mybir.EngineType.Unassigned)
            != mybir.EngineType.Unassigned
        ):
            waits = list(si.on_wait)
            keep = waits[: self.WAIT_CAP]
            extra = waits[self.WAIT_CAP :]
            si.on_wait[:] = keep
            for w in extra:
                nop = mybir.InstNoOp(
                    name=f"I-nw{self.nc.next_id()}",
                    engine=inst.engine,
                    bass_nofuse=True,
                    sync_info=mybir.SyncInfo(on_wait=[w], on_update=[]),
                )
                super()._commit_instruction(nop, lazy_reg_writes=False)
        return super()._commit_instruction(inst, lazy_reg_writes)

    def _drain_and_barrier(self, tick_clock, wait_clock):
        carrier = self.nc.sync.drain()
        wait_clock.add_sem_waits(
            carrier.ins, ScopedClock({None: tick_clock.global_clock})
        )
        si = carrier.ins.sync_info
        waits = list(si.on_wait) if si is not None else []
        if len(waits) > 1:
            si.on_wait[:] = waits[:1]
            for w in waits[1:]:
                extra = self.nc.sync.drain()
                extra.ins.sync_info = mybir.SyncInfo(on_wait=[w], on_update=[])
        self.nc.all_engine_barrier()
        assert self.sems is not None
        popped = self.nc._tile_sem_poison_stack.pop()
        assert popped is self._sem_poison
        self.nc.clear_and_free_semaphores(list(self.sems.allocated().values()))
        self.nc.all_engine_barrier()


def build_nc():
    nc = bass.Bass("TRN2", target_bir_lowering=False)
    xTb = nc.declare_dram_parameter("xTb", [E, C], BF16, isOutput=False)
    xT8 = nc.declare_dram_parameter("xT8", [E, C], FP8, isOutput=False)
    Wq8 = nc.declare_dram_parameter("Wq8", [E, GF], FP8, isOutput=False)
    Wk8 = nc.declare_dram_parameter("Wk8", [E, GF], FP8, isOutput=False)
    Wv = nc.declare_dram_parameter("Wv", [E, GF], BF16, isOutput=False)
    Wo = nc.declare_dram_parameter("Wo", [GF, E], BF16, isOutput=False)
    # triangular strip mask (k<=q within a 128x128 diagonal sub-block)
    mtri = nc.declare_dram_parameter("mtri", [128, KB], BF16, isOutput=False)
    # wide masks for qc=0 (zeros below the strip + triangle), width 128*(dr+1)
    mwide = nc.declare_dram_parameter("mwide", [128, 4 * QC], BF16, isOutput=False)
    out = nc.declare_dram_parameter("out", [C, E], F32, isOutput=True)

    xTb_t = xTb.ap().rearrange("(po pi) f -> pi po f", pi=128)  # [128, 8, C]
    xT8_t = xT8.ap().rearrange("(po pi) f -> pi po f", pi=128)
    Wq8_t = Wq8.ap().rearrange("(po pi) f -> pi po f", pi=128)  # [128, 8, 512]
    Wk8_t = Wk8.ap().rearrange("(po pi) f -> pi po f", pi=128)
    Wv_t = Wv.ap().rearrange("(po pi) f -> pi po f", pi=128)
    Wo_t = Wo.ap().rearrange("(po pi) f -> pi po f", pi=128)    # [128, 4, E]

    with PatchedTC(nc) as tc:
        import contextlib

        with contextlib.ExitStack() as ctx:
            consts = ctx.enter_context(tc.tile_pool(name="consts", bufs=1))
            ppsum = ctx.enter_context(tc.tile_pool(name="ppsum", bufs=2, space="PSUM"))
            xpool = ctx.enter_context(tc.tile_pool(name="xpool", bufs=1))
            vpool = ctx.enter_context(tc.tile_pool(name="vpool", bufs=1))
            qkpool = ctx.enter_context(tc.tile_pool(name="qkpool", bufs=1))
            stpool = ctx.enter_context(tc.tile_pool(name="stpsum", bufs=2, space="PSUM"))
            hidpool = ctx.enter_context(tc.tile_pool(name="hidpsum", bufs=1, space="PSUM"))
            wtpool = ctx.enter_context(tc.tile_pool(name="wtpool", bufs=3))
            napool = ctx.enter_context(tc.tile_pool(name="napool", bufs=2))
            hfpool = ctx.enter_context(tc.tile_pool(name="hfpool", bufs=1))
            ostage = ctx.enter_context(tc.tile_pool(name="ostage", bufs=3))

            mtri_sb = consts.tile([128, KB], BF16)
            mwide_sb = consts.tile([128, 4, QC], BF16)
            nc.sync.dma_start(mtri_sb[:], mtri.ap())
            nc.sync.dma_start(
                mwide_sb[:], mwide.ap().rearrange("p (r q) -> p r q", q=QC)
            )

            # DMA issue order tuned for startup: wv + first x_bf16 chunks
            # (V projections), then fp8 weights + x_fp8 (QK projections),
            # then the rest of x_bf16, then Wo (needed last).
            wv_sb = consts.tile([128, NE, GF], BF16)
            wq_sb = consts.tile([128, NE, GF], FP8)
            wk_sb = consts.tile([128, NE, GF], FP8)
            wo_sb = consts.tile([128, 4, E], BF16)
            x8_sb = xpool.tile([128, NE, C], FP8)
            xb_sb = xpool.tile([128, NE, C], BF16)
            nc.sync.dma_start(wv_sb[:], Wv_t[:])
            for t in range(4):
                nc.sync.dma_start(
                    xb_sb[:, :, t * KB : (t + 1) * KB],
                    xTb_t[:, :, t * KB : (t + 1) * KB],
                )
            nc.sync.dma_start(wq_sb[:], Wq8_t[:])
            nc.sync.dma_start(wk_sb[:], Wk8_t[:])
            for e in range(NE):
                nc.sync.dma_start(x8_sb[:, e, :], xT8_t[:, e, :])
            for t in range(4, NKB):
                nc.sync.dma_start(
                    xb_sb[:, :, t * KB : (t + 1) * KB],
                    xTb_t[:, :, t * KB : (t + 1) * KB],
                )
            nc.sync.dma_start(wo_sb[:], Wo_t[:])

            # v_sb: per k-block tile, 8 heads x 128 cols.  Even head of a
            # pair: [V(64) | ones(64)]; odd head: [ones | V].
            v_sb = vpool.tile([128, NKB, 2 * GF], BF16)
            nc.any.memset(v_sb[:], 1.0)

            # Q^T/K^T per head: [64 feats, C] fp8, every tile at base
            # partition 0 so consecutive score matmuls share PE rows and
            # run serially at full rate (row-disjoint fp8 pairs trigger a
            # chip-wide ~20% slowdown when run concurrently)
            qt = []
            kt = []
            for hh in range(8):
                qth = qkpool.tile([64, C], FP8, tag=f"qt{hh}", name=f"qt{hh}")
                kth = qkpool.tile([64, C], FP8, tag=f"kt{hh}", name=f"kt{hh}")
                qt.append(qth)
                kt.append(kth)

            hf = hfpool.tile([128, 4, C], BF16)  # hidT: 4 head-pairs x 64+64

            def emit_v(t):
                pv = ppsum.tile([128, GF], F32, tag="ppsum")
                for e in range(NE):
                    nc.tensor.matmul(
                        pv[:],
                        lhsT=xb_sb[:, e, t * KB : (t + 1) * KB],
                        rhs=wv_sb[:, e, :],
                        start=(e == 0),
                        stop=(e == NE - 1),
                    )
                src = pv[:].rearrange("p (h u) -> p h u", u=64)
                dst = v_sb[:, t, :].rearrange("p (h u) -> p h u", u=128)
                nc.vector.tensor_copy(dst[:, 0:8:2, 0:64], src[:, 0:8:2, :])
                nc.vector.tensor_copy(dst[:, 1:8:2, 64:128], src[:, 1:8:2, :])

            def emit_qk(hp, which, n):
                # one 512-token fp8-DoubleRow psum chain for head-pair hp
                # (DoubleRow pairs e-tiles: 4 matmuls contract all of E)
                w_sb = wq_sb if which == 0 else wk_sb
                dst = qt[2 * hp : 2 * hp + 2] if which == 0 else kt[2 * hp : 2 * hp + 2]
                pq = ppsum.tile([128, QC], F32, tag="ppsum")
                for i in range(4):
                    nc.tensor.matmul(
                        pq[:],
                        lhsT=w_sb[:, 2 * i : 2 * i + 2, hp * 128 : (hp + 1) * 128],
                        rhs=x8_sb[:, 2 * i : 2 * i + 2, n * QC : (n + 1) * QC],
                        start=(i == 0),
                        stop=(i == 3),
                        perf_mode=DR,
                    )
                nc.vector.tensor_copy(dst[0][:, n * QC : (n + 1) * QC], pq[0:64, :])
                nc.vector.tensor_copy(dst[1][:, n * QC : (n + 1) * QC], pq[64:128, :])

            def emit_o(qb):
                for ec in range(E // QC):
                    po = ppsum.tile([128, QC], F32, tag="ppsum")
                    for f in range(4):
                        nc.tensor.matmul(
                            po[:],
                            lhsT=hf[:, f, qb * KB : (qb + 1) * KB],
                            rhs=wo_sb[:, f, ec * QC : (ec + 1) * QC],
                            start=(f == 0),
                            stop=(f == 3),
                        )
                    so = ostage.tile([128, QC], F32, tag="so")
                    nc.vector.tensor_copy(so[:], po[:])
                    nc.sync.dma_start(
                        out.ap()[qb * KB : (qb + 1) * KB, ec * QC : (ec + 1) * QC],
                        so[:],
                    )

            # ---- filler work queue --------------------------------------
            # entries: (ready_step, deadline_step, emit_fn).  A filler may
            # only be EMITTED at sweep position >= ready (tile-framework deps
            # are tracked in emission order, so e.g. an O-projection emitted
            # before its q-chunk's hidT writes would read unwritten data);
            # it MUST be emitted before position == deadline to avoid stalls.
            sweep = [
                (0, 0), (0, 1), (1, 0), (1, 1),
                (0, 2), (0, 3), (1, 2), (1, 3),
                (2, 0), (2, 1), (2, 2), (2, 3),
                (3, 0), (3, 1), (3, 2), (3, 3),
            ]
            qc_done_after = {0: 5, 1: 7, 2: 11, 3: 15}

            fq = []
            for n in range(NQC):
                for which in range(2):
                    fq.append((0, 1, lambda n=n, w=which: emit_qk(1, w, n)))
            for t in range(4, 8):
                fq.append((0, 2, lambda t=t: emit_v(t)))
            for hp in (2, 3):
                for n in range(NQC):
                    for which in range(2):
                        fq.append(
                            (0, 4, lambda hp=hp, n=n, w=which: emit_qk(hp, w, n))
                        )
            for t in range(8, 12):
                fq.append((0, 8, lambda t=t: emit_v(t)))
            for t in range(12, 16):
                fq.append((0, 12, lambda t=t: emit_v(t)))
            for qcj in range(3):
                for qb in range(4 * qcj, 4 * qcj + 4):
                    fq.append(
                        (qc_done_after[qcj] + 1, 16, lambda qb=qb: emit_o(qb))
                    )

            # ---- phase A: V(0..3) + head-pair 0 QK projections ----------
            for t in range(4):
                emit_v(t)
            for n in range(NQC):
                for which in range(2):
                    emit_qk(0, which, n)

            # ---- phase B: attention sweep with interleaved fillers ------
            def pop_due(step):
                # force-emit everything whose deadline has arrived
                i = 0
                while i < len(fq):
                    ready, deadline, fn = fq[i]
                    if deadline <= step:
                        assert ready <= step
                        fq.pop(i)
                        fn()
                    else:
                        i += 1

            def pop_one(step):
                for i, (ready, deadline, fn) in enumerate(fq):
                    if ready <= step:
                        fq.pop(i)
                        fn()
                        return

            pending_norm = [None]

            def emit_norm(hidA, hidB, hp, qc):
                # 1/rowsum via exp(-ln(rs)) on ACT; hidA rowsum at rows
                # 64:128, hidB ([ones|V]) rowsum at rows 0:64.
                lnA = napool.tile([64, QC], F32, tag="ln")
                recA = napool.tile([64, QC], F32, tag="rec")
                lnB = napool.tile([64, QC], F32, tag="ln")
                recB = napool.tile([64, QC], F32, tag="rec")
                nc.scalar.activation(
                    lnA[:], hidA[64:128, :], mybir.ActivationFunctionType.Ln
                )
                nc.scalar.activation(
                    recA[:], lnA[:], mybir.ActivationFunctionType.Exp, scale=-1.0
                )
                nc.scalar.activation(
                    lnB[:], hidB[0:64, :], mybir.ActivationFunctionType.Ln
                )
                nc.scalar.activation(
                    recB[:], lnB[:], mybir.ActivationFunctionType.Exp, scale=-1.0
                )
                nc.vector.tensor_tensor(
                    hf[0:64, hp, qc * QC : (qc + 1) * QC],
                    hidA[0:64, :],
                    recA[:],
                    mybir.AluOpType.mult,
                )
                nc.vector.tensor_tensor(
                    hf[64:128, hp, qc * QC : (qc + 1) * QC],
                    hidB[64:128, :],
                    recB[:],
                    mybir.AluOpType.mult,
                )

            for step, (qc, hp) in enumerate(sweep):
                # anything whose deadline is this step must be emitted now
                pop_due(step)
                nkb = 4 * qc + 4
                # diag-first order: full-width dr0 opens the accumulation,
                # a full-width block closes it (qc=0 runs untrimmed)
                kbs = [nkb - 4 + dr for dr in range(4)] + list(range(nkb - 4))

                def s_off(kb):
                    dr = kb - (nkb - 4)
                    return 128 * dr if (dr >= 0 and qc > 0) else 0

                def emit_s(kb):
                    off = s_off(kb)
                    st = stpool.tile([128, 2, QC], F32, tag="st")
                    for h in range(2):
                        nc.tensor.matmul(
                            st[:, h, off:QC],
                            lhsT=kt[2 * hp + h][:, kb * KB : (kb + 1) * KB],
                            rhs=qt[2 * hp + h][:, qc * QC + off : (qc + 1) * QC],
                            start=True,
                            stop=True,
                        )
                    return st

                hidA = hidpool.tile([128, QC], F32, tag="hidA")
                hidB = hidpool.tile([128, QC], F32, tag="hidB")
                # scores lead the exp->mask->hid chain by one block so the
                # ACT exp stream never waits on the tensor engine
                st = emit_s(kbs[0])
                if pending_norm[0] is not None:
                    pending_norm[0]()
                    pending_norm[0] = None
                for j, kb in enumerate(kbs):
                    dr = kb - (nkb - 4)
                    diag = dr >= 0
                    off = s_off(kb)
                    wt = wtpool.tile([128, 2, QC], BF16, tag="wt")
                    eoff = 0 if qc == 0 else off
                    nc.scalar.activation(
                        wt[:, :, eoff:QC],
                        st[:, :, eoff:QC],
                        mybir.ActivationFunctionType.Exp,
                        scale=EXPSCALE,
                    )
                    if diag:
                        if qc == 0:
                            # zeros below the strip + triangle, width 128(dr+1)
                            wdr = 128 * (dr + 1)
                            nc.vector.tensor_tensor(
                                wt[:, :, 0:wdr],
                                wt[:, :, 0:wdr],
                                mwide_sb[:, None, dr, 0:wdr].to_broadcast(
                                    (128, 2, wdr)
                                ),
                                mybir.AluOpType.mult,
                            )
                        else:
                            nc.vector.tensor_tensor(
                                wt[:, :, off : off + KB],
                                wt[:, :, off : off + KB],
                                mtri_sb[:, None, :].to_broadcast((128, 2, KB)),
                                mybir.AluOpType.mult,
                            )
                    if j + 1 < len(kbs):
                        st = emit_s(kbs[j + 1])
                    first, last = (j == 0), (j == len(kbs) - 1)
                    nc.tensor.matmul(
                        hidA[:, off:QC],
                        lhsT=v_sb[:, kb, (2 * hp) * KB : (2 * hp + 1) * KB],
                        rhs=wt[:, 0, off:QC],
                        start=first,
                        stop=last,
                    )
                    nc.tensor.matmul(
                        hidB[:, off:QC],
                        lhsT=v_sb[:, kb, (2 * hp + 1) * KB : (2 * hp + 2) * KB],
                        rhs=wt[:, 1, off:QC],
                        start=first,
                        stop=last,
                    )
                    pop_one(step)
                pending_norm[0] = (
                    lambda a=hidA, b=hidB, hp=hp, qc=qc: emit_norm(a, b, hp, qc)
                )
            pending_norm[0]()

            # drain remaining fillers + final q-chunk's output projection
            while fq:
                fq.pop(0)[2]()
            for qb in range(12, 16):
                emit_o(qb)
    return nc


def _make_masks():
    import ml_dtypes

    kk = np.arange(128)[:, None]
    tri = (kk <= np.arange(KB)[None, :]).astype(np.float32)
    wide = np.zeros((128, 4, QC), dtype=np.float32)
    for dr in range(4):
        qq = np.arange(QC)[None, :]
        wide[:, dr, :] = (128 * dr + kk <= qq).astype(np.float32)
    return (
        tri.astype(ml_dtypes.bfloat16),
        np.ascontiguousarray(wide.reshape(128, 4 * QC)).astype(ml_dtypes.bfloat16),
    )


def make_in_maps(x, W_q, W_k, W_v, W_o):
    import ml_dtypes

    bf16 = ml_dtypes.bfloat16
    e4 = ml_dtypes.float8_e4m3fn
    mtri, mwide = _make_masks()
    in_maps = []
    for i in range(N_CORES):
        b, g = i // 2, i % 2
        xT = np.ascontiguousarray(np.asarray(x)[b].T)
        wq = np.asarray(W_q)[:, g * GF : (g + 1) * GF] * SC
        wk = np.asarray(W_k)[:, g * GF : (g + 1) * GF] * SC
        in_maps.append(
            {
                "xTb": xT.astype(bf16),
                "xT8": xT.astype(e4),
                "Wq8": np.ascontiguousarray(wq).astype(e4),
                "Wk8": np.ascontiguousarray(wk).astype(e4),
                "Wv": np.ascontiguousarray(
                    np.asarray(W_v)[:, g * GF : (g + 1) * GF]
                ).astype(bf16),
                "Wo": np.ascontiguousarray(
                    np.asarray(W_o)[g * GF : (g + 1) * GF, :]
                ).astype(bf16),
                "mtri": mtri,
                "mwide": mwide,
            }
        )
    return in_maps


def kernel(x, W_q, W_k, W_v, W_o):
    global _CACHED_NC
    from concourse.bass_utils import run_bass_kernel_spmd

    if _CACHED_NC is None:
        _CACHED_NC = build_nc()
    nc = _CACHED_NC

    in_maps = make_in_maps(x, W_q, W_k, W_v, W_o)
    res = run_bass_kernel_spmd(nc, in_maps, core_ids=list(range(N_CORES)))
    out = np.empty((B, C, E), dtype=np.float32)
    for b in range(B):
        out[b] = res.results[2 * b]["out"] + res.results[2 * b + 1]["out"]
    return out


# revision 17
# speedup vs baseline: 1.3071x; 1.3071x over previous
"""Multi-head causal attention (B=4, C=2048, E=1024, H=16, D=64) on 8 trn2 cores.

Sharding: core i = (batch b=i//2, head-group g=i%2).  Each core computes its
batch's attention for 8 heads (512 features) and a partial output projection;
the host sums the two partials per batch (W_o split row-wise).

Per-core pipeline (ACT-exp is the pacing engine; tensor work is interleaved
into its gaps):
  - Q/K projections in fp8e4 DoubleRow (x and W_q/W_k host-quantized to fp8;
    W_q/W_k host-scaled by 16 to avoid fp8 subnormals and column-permuted so
    the psum partition layout is the packed [4 heads x 32 feats] x 2 slabs
    needed by the score matmuls).  4 DR matmuls per 512-token psum.
  - Scores S^T per head: fp8 DoubleRow, lhsT/rhs = 32-partition slices of the
    packed K^T/Q^T tiles (contraction 64 = 32 partitions x 2 slabs).
    Diagonal q-chunks are column-trimmed (only q >= k-block start computed).
  - exp on ACT with scale 1/2048 (scores carry the 16x16 weight scaling),
    output bf16 wt; triangular mask multiply on DVE for diagonal blocks only.
  - hid = [V|1s]^T @ wt in bf16, accumulated over k-blocks diag-first so the
    trimmed column ranges have a full-width first (start) and last (stop)
    matmul.  Even head of a pair uses [V|ones] (hid rows 0:64, rowsum rows
    64:128); odd head uses [ones|V] so its normalize multiply can write hidT
    rows 64:128 directly (DVE requires aligned SBUF operands; one PSUM input
    with a shifted SBUF operand is allowed).
  - 1/rowsum via exp(-ln(rs)) on ACT (reciprocal's table doesn't coexist with
    exp's; ln/exp share one set), then DVE mult psum-hid x recip -> hidT SBUF.
  - out = hidT^T @ W_o in bf16, interleaved per 4-qb chunk as soon as the
    previous q-chunk's hidT completes; psum -> f32 staging -> DRAM.
Scheduling: qc-outer sweep ordered so quad0's first two q-chunks run while
quad1's QK projections (and streamed V projections) fill the tensor gaps.
"""

import numpy as np

import concourse.bass as bass
import concourse.mybir as mybir
import concourse.tile as tile
from concourse.vector_clock import ScopedClock

B, C, E = 4, 2048, 1024
H, D = 16, 64
N_CORES = 8
GF = 512          # features per head-group (8 heads x 64)
QC = 512          # q-chunk width
KB = 128          # k-block width
NQC = C // QC     # 4
NKB = C // KB     # 16
NE = E // 128     # 8 contraction tiles over E
F32 = mybir.dt.float32
BF16 = mybir.dt.bfloat16
FP8 = mybir.dt.float8e4
F32R = mybir.dt.float32r
DR = mybir.MatmulPerfMode.DoubleRow
SC = 16.0                   # host scale on W_q/W_k (fp8 subnormal avoidance)
EXPSCALE = 0.125 / (SC * SC)  # 1/sqrt(D) / (16*16)

_CACHED_NC = None


class PatchedTC(tile.TileContext):
    """This walrus build caps sync waits per instruction (1 for CTRL, ~2 for
    compute ISA structs).  Hoist excess waits onto same-engine NOPs emitted
    just before the instruction (engine streams execute in order, so the
    semantics are identical), and split the end-of-kernel drain's waits
    across single-wait drain instructions."""

    WAIT_CAP = 1

    def _commit_instruction(self, inst, lazy_reg_writes=True):
        si = getattr(inst, "sync_info", None)
        if (
            si is not None
            and len(si.on_wait) > self.WAIT_CAP
            and getattr(inst, "engine", mybir.EngineType.Unassigned)
            != mybir.EngineType.Unassigned
        ):
            waits = list(si.on_wait)
            keep = waits[: self.WAIT_CAP]
            extra = waits[self.WAIT_CAP :]
            si.on_wait[:] = keep
            for w in extra:
                nop = mybir.InstNoOp(
                    name=f"I-nw{self.nc.next_id()}",
                    engine=inst.engine,
                    bass_nofuse=True,
                    sync_info=mybir.SyncInfo(on_wait=[w], on_update=[]),
                )
                super()._commit_instruction(nop, lazy_reg_writes=False)
        return super()._commit_instruction(inst, lazy_reg_writes)

    def _drain_and_barrier(self, tick_clock, wait_clock):
        carrier = self.nc.sync.drain()
        wait_clock.add_sem_waits(
            carrier.ins, ScopedClock({None: tick_clock.global_clock})
        )
        si = carrier.ins.sync_info
        waits = list(si.on_wait) if si is not None else []
        if len(waits) > 1:
            si.on_wait[:] = waits[:1]
            for w in waits[1:]:
                extra = self.nc.sync.drain()
                extra.ins.sync_info = mybir.SyncInfo(on_wait=[w], on_update=[])
        self.nc.all_engine_barrier()
        assert self.sems is not None
        popped = self.nc._tile_sem_poison_stack.pop()
        assert popped is self._sem_poison
        self.nc.clear_and_free_semaphores(list(self.sems.allocated().values()))
        self.nc.all_engine_barrier()


def build_nc():
    nc = bass.Bass("TRN2", target_bir_lowering=False)
    xTb = nc.declare_dram_parameter("xTb", [E, C], BF16, isOutput=False)
    xT8 = nc.declare_dram_parameter("xT8", [E, C], FP8, isOutput=False)
    Wq8 = nc.declare_dram_parameter("Wq8", [E, GF], FP8, isOutput=False)
    Wk8 = nc.declare_dram_parameter("Wk8", [E, GF], FP8, isOutput=False)
    Wv = nc.declare_dram_parameter("Wv", [E, GF], BF16, isOutput=False)
    Wo = nc.declare_dram_parameter("Wo", [GF, E], BF16, isOutput=False)
    # triangular strip mask (k<=q within a 128x128 diagonal sub-block)
    mtri = nc.declare_dram_parameter("mtri", [128, KB], BF16, isOutput=False)
    # wide masks for qc=0 (zeros below the strip + triangle), width 128*(dr+1)
    mwide = nc.declare_dram_parameter("mwide", [128, 4 * QC], BF16, isOutput=False)
    out = nc.declare_dram_parameter("out", [C, E], F32, isOutput=True)

    xTb_t = xTb.ap().rearrange("(po pi) f -> pi po f", pi=128)  # [128, 8, C]
    xT8_t = xT8.ap().rearrange("(po pi) f -> pi po f", pi=128)
    Wq8_t = Wq8.ap().rearrange("(po pi) f -> pi po f", pi=128)  # [128, 8, 512]
    Wk8_t = Wk8.ap().rearrange("(po pi) f -> pi po f", pi=128)
    Wv_t = Wv.ap().rearrange("(po pi) f -> pi po f", pi=128)
    Wo_t = Wo.ap().rearrange("(po pi) f -> pi po f", pi=128)    # [128, 4, E]

    with PatchedTC(nc) as tc:
        import contextlib

        with contextlib.ExitStack() as ctx:
            consts = ctx.enter_context(tc.tile_pool(name="consts", bufs=1))
            ppsum = ctx.enter_context(tc.tile_pool(name="ppsum", bufs=2, space="PSUM"))
            xpool = ctx.enter_context(tc.tile_pool(name="xpool", bufs=1))
            vpool = ctx.enter_context(tc.tile_pool(name="vpool", bufs=1))
            qkpool = ctx.enter_context(tc.tile_pool(name="qkpool", bufs=1))
            stpool = ctx.enter_context(tc.tile_pool(name="stpsum", bufs=2, space="PSUM"))
            hidpool = ctx.enter_context(tc.tile_pool(name="hidpsum", bufs=1, space="PSUM"))
            wtpool = ctx.enter_context(tc.tile_pool(name="wtpool", bufs=2))
            napool = ctx.enter_context(tc.tile_pool(name="napool", bufs=2))
            hfpool = ctx.enter_context(tc.tile_pool(name="hfpool", bufs=1))
            ostage = ctx.enter_context(tc.tile_pool(name="ostage", bufs=2))

            mtri_sb = consts.tile([128, KB], BF16)
            mwide_sb = consts.tile([128, 4, QC], BF16)
            nc.sync.dma_start(mtri_sb[:], mtri.ap())
            nc.sync.dma_start(
                mwide_sb[:], mwide.ap().rearrange("p (r q) -> p r q", q=QC)
            )

            # DMA issue order tuned for startup: wv + first x_bf16 chunks
            # (V projections), then fp8 weights + x_fp8 (QK projections),
            # then the rest of x_bf16, then Wo (needed last).
            wv_sb = consts.tile([128, NE, GF], BF16)
            wq_sb = consts.tile([128, NE, GF], FP8)
            wk_sb = consts.tile([128, NE, GF], FP8)
            wo_sb = consts.tile([128, 4, E], BF16)
            x8_sb = xpool.tile([128, NE, C], FP8)
            xb_sb = xpool.tile([128, NE, C], BF16)
            nc.sync.dma_start(wv_sb[:], Wv_t[:])
            for t in range(4):
                nc.sync.dma_start(
                    xb_sb[:, :, t * KB : (t + 1) * KB],
                    xTb_t[:, :, t * KB : (t + 1) * KB],
                )
            nc.sync.dma_start(wq_sb[:], Wq8_t[:])
            nc.sync.dma_start(wk_sb[:], Wk8_t[:])
            for e in range(NE):
                nc.sync.dma_start(x8_sb[:, e, :], xT8_t[:, e, :])
            for t in range(4, NKB):
                nc.sync.dma_start(
                    xb_sb[:, :, t * KB : (t + 1) * KB],
                    xTb_t[:, :, t * KB : (t + 1) * KB],
                )
            nc.sync.dma_start(wo_sb[:], Wo_t[:])

            # v_sb: per k-block tile, 8 heads x 128 cols.  Even head of a
            # pair: [V(64) | ones(64)]; odd head: [ones | V].
            v_sb = vpool.tile([128, NKB, 2 * GF], BF16)
            nc.any.memset(v_sb[:], 1.0)

            # Q^T/K^T per head-pair: [128, C] f32r (head A rows 0:64,
            # head B rows 64:128).  Plain fp8 score matmuls measurably slow
            # the whole chip ~20% (f32r row-tiled pairs, as in the original
            # kernel, run at full clock), so scores stay f32r.
            qt = []
            kt = []
            for pp in range(4):
                qtp = qkpool.tile([128, C], F32R, tag=f"qt{pp}", name=f"qt{pp}")
                ktp = qkpool.tile([128, C], F32R, tag=f"kt{pp}", name=f"kt{pp}")
                qt.append(qtp)
                kt.append(ktp)

            hf = hfpool.tile([128, 4, C], BF16)  # hidT: 4 head-pairs x 64+64

            def emit_v(t):
                pv = ppsum.tile([128, GF], F32, tag="ppsum")
                for e in range(NE):
                    nc.tensor.matmul(
                        pv[:],
                        lhsT=xb_sb[:, e, t * KB : (t + 1) * KB],
                        rhs=wv_sb[:, e, :],
                        start=(e == 0),
                        stop=(e == NE - 1),
                    )
                src = pv[:].rearrange("p (h u) -> p h u", u=64)
                dst = v_sb[:, t, :].rearrange("p (h u) -> p h u", u=128)
                nc.vector.tensor_copy(dst[:, 0:8:2, 0:64], src[:, 0:8:2, :])
                nc.vector.tensor_copy(dst[:, 1:8:2, 64:128], src[:, 1:8:2, :])

            def emit_qk(hp, which, n):
                # one 512-token fp8-DoubleRow psum chain for head-pair hp
                # (DoubleRow pairs e-tiles: 4 matmuls contract all of E)
                w_sb = wq_sb if which == 0 else wk_sb
                dst = qt[hp] if which == 0 else kt[hp]
                pq = ppsum.tile([128, QC], F32, tag="ppsum")
                for i in range(4):
                    nc.tensor.matmul(
                        pq[:],
                        lhsT=w_sb[:, 2 * i : 2 * i + 2, hp * 128 : (hp + 1) * 128],
                        rhs=x8_sb[:, 2 * i : 2 * i + 2, n * QC : (n + 1) * QC],
                        start=(i == 0),
                        stop=(i == 3),
                        perf_mode=DR,
                    )
                nc.vector.tensor_copy(dst[:, n * QC : (n + 1) * QC], pq[:])

            def emit_o(qb):
                for ec in range(E // QC):
                    po = ppsum.tile([128, QC], F32, tag="ppsum")
                    for f in range(4):
                        nc.tensor.matmul(
                            po[:],
                            lhsT=hf[:, f, qb * KB : (qb + 1) * KB],
                            rhs=wo_sb[:, f, ec * QC : (ec + 1) * QC],
                            start=(f == 0),
                            stop=(f == 3),
                        )
                    so = ostage.tile([128, QC], F32, tag="so")
                    nc.vector.tensor_copy(so[:], po[:])
                    nc.sync.dma_start(
                        out.ap()[qb * KB : (qb + 1) * KB, ec * QC : (ec + 1) * QC],
                        so[:],
                    )

            # ---- filler work queue --------------------------------------
            # entries: (ready_step, deadline_step, emit_fn).  A filler may
            # only be EMITTED at sweep position >= ready (tile-framework deps
            # are tracked in emission order, so e.g. an O-projection emitted
            # before its q-chunk's hidT writes would read unwritten data);
            # it MUST be emitted before position == deadline to avoid stalls.
            sweep = [
                (0, 0), (0, 1), (1, 0), (1, 1),
                (0, 2), (0, 3), (1, 2), (1, 3),
                (2, 0), (2, 1), (2, 2), (2, 3),
                (3, 0), (3, 1), (3, 2), (3, 3),
            ]
            qc_done_after = {0: 5, 1: 7, 2: 11, 3: 15}

            fq = []
            for n in range(NQC):
                for which in range(2):
                    fq.append((0, 1, lambda n=n, w=which: emit_qk(1, w, n)))
            for t in range(4, 8):
                fq.append((0, 2, lambda t=t: emit_v(t)))
            for hp in (2, 3):
                for n in range(NQC):
                    for which in range(2):
                        fq.append(
                            (0, 4, lambda hp=hp, n=n, w=which: emit_qk(hp, w, n))
                        )
            for t in range(8, 12):
                fq.append((0, 8, lambda t=t: emit_v(t)))
            for t in range(12, 16):
                fq.append((0, 12, lambda t=t: emit_v(t)))
            for qcj in range(3):
                for qb in range(4 * qcj, 4 * qcj + 4):
                    fq.append(
                        (qc_done_after[qcj] + 1, 16, lambda qb=qb: emit_o(qb))
                    )

            # ---- phase A: V(0..3) + head-pair 0 QK projections ----------
            for t in range(4):
                emit_v(t)
            for n in range(NQC):
                for which in range(2):
                    emit_qk(0, which, n)

            # ---- phase B: attention sweep with interleaved fillers ------
            def pop_due(step):
                # force-emit everything whose deadline has arrived
                i = 0
                while i < len(fq):
                    ready, deadline, fn = fq[i]
                    if deadline <= step:
                        assert ready <= step
                        fq.pop(i)
                        fn()
                    else:
                        i += 1

            def pop_one(step):
                for i, (ready, deadline, fn) in enumerate(fq):
                    if ready <= step:
                        fq.pop(i)
                        fn()
                        return

            pending_norm = [None]

            def emit_norm(hidA, hidB, hp, qc):
                # 1/rowsum via exp(-ln(rs)) on ACT; hidA rowsum at rows
                # 64:128, hidB ([ones|V]) rowsum at rows 0:64.
                lnA = napool.tile([64, QC], F32, tag="ln")
                recA = napool.tile([64, QC], F32, tag="rec")
                lnB = napool.tile([64, QC], F32, tag="ln")
                recB = napool.tile([64, QC], F32, tag="rec")
                nc.scalar.activation(
                    lnA[:], hidA[64:128, :], mybir.ActivationFunctionType.Ln
                )
                nc.scalar.activation(
                    recA[:], lnA[:], mybir.ActivationFunctionType.Exp, scale=-1.0
                )
                nc.scalar.activation(
                    lnB[:], hidB[0:64, :], mybir.ActivationFunctionType.Ln
                )
                nc.scalar.activation(
                    recB[:], lnB[:], mybir.ActivationFunctionType.Exp, scale=-1.0
                )
                nc.vector.tensor_tensor(
                    hf[0:64, hp, qc * QC : (qc + 1) * QC],
                    hidA[0:64, :],
                    recA[:],
                    mybir.AluOpType.mult,
                )
                nc.vector.tensor_tensor(
                    hf[64:128, hp, qc * QC : (qc + 1) * QC],
                    hidB[64:128, :],
                    recB[:],
                    mybir.AluOpType.mult,
                )

            for step, (qc, hp) in enumerate(sweep):
                # anything whose deadline is this step must be emitted now
                pop_due(step)
                nkb = 4 * qc + 4
                # diag-first order: full-width dr0 opens the accumulation,
                # a full-width block closes it (qc=0 runs untrimmed)
                kbs = [nkb - 4 + dr for dr in range(4)] + list(range(nkb - 4))

                def s_off(kb):
                    dr = kb - (nkb - 4)
                    return 128 * dr if (dr >= 0 and qc > 0) else 0

                def emit_s(kb):
                    off = s_off(kb)
                    st = stpool.tile([128, 2, QC], F32, tag="st")
                    for h in range(2):
                        nc.tensor.matmul(
                            st[:, h, off:QC],
                            lhsT=kt[hp][64 * h : 64 * h + 64, kb * KB : (kb + 1) * KB],
                            rhs=qt[hp][64 * h : 64 * h + 64, qc * QC + off : (qc + 1) * QC],
                            start=True,
                            stop=True,
                        )
                    return st

                hidA = hidpool.tile([128, QC], F32, tag="hidA")
                hidB = hidpool.tile([128, QC], F32, tag="hidB")
                # scores lead the exp->mask->hid chain by one block so the
                # ACT exp stream never waits on the tensor engine
                st = emit_s(kbs[0])
                if pending_norm[0] is not None:
                    pending_norm[0]()
                    pending_norm[0] = None
                for j, kb in enumerate(kbs):
                    dr = kb - (nkb - 4)
                    diag = dr >= 0
                    off = s_off(kb)
                    wt = wtpool.tile([128, 2, QC], BF16, tag="wt")
                    eoff = 0 if qc == 0 else off
                    nc.scalar.activation(
                        wt[:, :, eoff:QC],
                        st[:, :, eoff:QC],
                        mybir.ActivationFunctionType.Exp,
                        scale=EXPSCALE,
                    )
                    if diag:
                        if qc == 0:
                            # zeros below the strip + triangle, width 128(dr+1)
                            wdr = 128 * (dr + 1)
                            nc.vector.tensor_tensor(
                                wt[:, :, 0:wdr],
                                wt[:, :, 0:wdr],
                                mwide_sb[:, None, dr, 0:wdr].to_broadcast(
                                    (128, 2, wdr)
                                ),
                                mybir.AluOpType.mult,
                            )
                        else:
                            nc.vector.tensor_tensor(
                                wt[:, :, off : off + KB],
                                wt[:, :, off : off + KB],
                                mtri_sb[:, None, :].to_broadcast((128, 2, KB)),
                                mybir.AluOpType.mult,
                            )
                    if j + 1 < len(kbs):
                        st = emit_s(kbs[j + 1])
                    first, last = (j == 0), (j == len(kbs) - 1)
                    nc.tensor.matmul(
                        hidA[:, off:QC],
                        lhsT=v_sb[:, kb, (2 * hp) * KB : (2 * hp + 1) * KB],
                        rhs=wt[:, 0, off:QC],
                        start=first,
                        stop=last,
                    )
                    nc.tensor.matmul(
                        hidB[:, off:QC],
                        lhsT=v_sb[:, kb, (2 * hp + 1) * KB : (2 * hp + 2) * KB],
                        rhs=wt[:, 1, off:QC],
                        start=first,
                        stop=last,
                    )
                    pop_one(step)
                pending_norm[0] = (
                    lambda a=hidA, b=hidB, hp=hp, qc=qc: emit_norm(a, b, hp, qc)
                )
            pending_norm[0]()

            # drain remaining fillers + final q-chunk's output projection
            while fq:
                fq.pop(0)[2]()
            for qb in range(12, 16):
                emit_o(qb)
    return nc


def _make_masks():
    import ml_dtypes

    kk = np.arange(128)[:, None]
    tri = (kk <= np.arange(KB)[None, :]).astype(np.float32)
    wide = np.zeros((128, 4, QC), dtype=np.float32)
    for dr in range(4):
        qq = np.arange(QC)[None, :]
        wide[:, dr, :] = (128 * dr + kk <= qq).astype(np.float32)
    return (
        tri.astype(ml_dtypes.bfloat16),
        np.ascontiguousarray(wide.reshape(128, 4 * QC)).astype(ml_dtypes.bfloat16),
    )


def make_in_maps(x, W_q, W_k, W_v, W_o):
    import ml_dtypes

    bf16 = ml_dtypes.bfloat16
    e4 = ml_dtypes.float8_e4m3fn
    mtri, mwide = _make_masks()
    in_maps = []
    for i in range(N_CORES):
        b, g = i // 2, i % 2
        xT = np.ascontiguousarray(np.asarray(x)[b].T)
        wq = np.asarray(W_q)[:, g * GF : (g + 1) * GF] * SC
        wk = np.asarray(W_k)[:, g * GF : (g + 1) * GF] * SC
        in_maps.append(
            {
                "xTb": xT.astype(bf16),
                "xT8": xT.astype(e4),
                "Wq8": np.ascontiguousarray(wq).astype(e4),
                "Wk8": np.ascontiguousarray(wk).astype(e4),
                "Wv": np.ascontiguousarray(
                    np.asarray(W_v)[:, g * GF : (g + 1) * GF]
                ).astype(bf16),
                "Wo": np.ascontiguousarray(
                    np.asarray(W_o)[g * GF : (g + 1) * GF, :]
                ).astype(bf16),
                "mtri": mtri,
                "mwide": mwide,
            }
        )
    return in_maps


def kernel(x, W_q, W_k, W_v, W_o):
    global _CACHED_NC
    from concourse.bass_utils import run_bass_kernel_spmd

    if _CACHED_NC is None:
        _CACHED_NC = build_nc()
    nc = _CACHED_NC

    in_maps = make_in_maps(x, W_q, W_k, W_v, W_o)
    res = run_bass_kernel_spmd(nc, in_maps, core_ids=list(range(N_CORES)))
    out = np.empty((B, C, E), dtype=np.float32)
    for b in range(B):
        out[b] = res.results[2 * b]["out"] + res.results[2 * b + 1]["out"]
    return out


# revision 18
# speedup vs baseline: 1.3496x; 1.0325x over previous
"""Multi-head causal attention (B=4, C=2048, E=1024, H=16, D=64) on 8 trn2 cores.

Sharding: core i = (batch b=i//2, head-group g=i%2).  Each core computes its
batch's attention for 8 heads (512 features) and a partial output projection;
the host sums the two partials per batch (W_o split row-wise).

Per-core pipeline (ACT-exp is the pacing engine; tensor work is interleaved
into its gaps):
  - Q/K projections in fp8e4 DoubleRow (x and W_q/W_k host-quantized to fp8;
    W_q/W_k host-scaled by 16 to avoid fp8 subnormals and column-permuted so
    the psum partition layout is the packed [4 heads x 32 feats] x 2 slabs
    needed by the score matmuls).  4 DR matmuls per 512-token psum.
  - Scores S^T per head: fp8 DoubleRow, lhsT/rhs = 32-partition slices of the
    packed K^T/Q^T tiles (contraction 64 = 32 partitions x 2 slabs).
    Diagonal q-chunks are column-trimmed (only q >= k-block start computed).
  - exp on ACT with scale 1/2048 (scores carry the 16x16 weight scaling),
    output bf16 wt; triangular mask multiply on DVE for diagonal blocks only.
  - hid = [V|1s]^T @ wt in bf16, accumulated over k-blocks diag-first so the
    trimmed column ranges have a full-width first (start) and last (stop)
    matmul.  Even head of a pair uses [V|ones] (hid rows 0:64, rowsum rows
    64:128); odd head uses [ones|V] so its normalize multiply can write hidT
    rows 64:128 directly (DVE requires aligned SBUF operands; one PSUM input
    with a shifted SBUF operand is allowed).
  - 1/rowsum via exp(-ln(rs)) on ACT (reciprocal's table doesn't coexist with
    exp's; ln/exp share one set), then DVE mult psum-hid x recip -> hidT SBUF.
  - out = hidT^T @ W_o in bf16, interleaved per 4-qb chunk as soon as the
    previous q-chunk's hidT completes; psum -> f32 staging -> DRAM.
Scheduling: qc-outer sweep ordered so quad0's first two q-chunks run while
quad1's QK projections (and streamed V projections) fill the tensor gaps.
"""

import numpy as np

import concourse.bass as bass
import concourse.mybir as mybir
import concourse.tile as tile
from concourse.vector_clock import ScopedClock

B, C, E = 4, 2048, 1024
H, D = 16, 64
N_CORES = 8
GF = 512          # features per head-group (8 heads x 64)
QC = 512          # q-chunk width
KB = 128          # k-block width
NQC = C // QC     # 4
NKB = C // KB     # 16
NE = E // 128     # 8 contraction tiles over E
F32 = mybir.dt.float32
BF16 = mybir.dt.bfloat16
FP8 = mybir.dt.float8e4
F32R = mybir.dt.float32r
DR = mybir.MatmulPerfMode.DoubleRow
SC = 16.0                   # host scale on W_q/W_k (fp8 subnormal avoidance)
EXPSCALE = 0.125 / (SC * SC)  # 1/sqrt(D) / (16*16)

_CACHED_NC = None


class PatchedTC(tile.TileContext):
    """This walrus build caps sync waits per instruction (1 for CTRL, ~2 for
    compute ISA structs).  Hoist excess waits onto same-engine NOPs emitted
    just before the instruction (engine streams execute in order, so the
    semantics are identical), and split the end-of-kernel drain's waits
    across single-wait drain instructions."""

    WAIT_CAP = 1

    def _commit_instruction(self, inst, lazy_reg_writes=True):
        si = getattr(inst, "sync_info", None)
        if (
            si is not None
            and len(si.on_wait) > self.WAIT_CAP
            and getattr(inst, "engine", mybir.EngineType.Unassigned)
            != mybir.EngineType.Unassigned
        ):
            waits = list(si.on_wait)
            keep = waits[: self.WAIT_CAP]
            extra = waits[self.WAIT_CAP :]
            si.on_wait[:] = keep
            for w in extra:
                nop = mybir.InstNoOp(
                    name=f"I-nw{self.nc.next_id()}",
                    engine=inst.engine,
                    bass_nofuse=True,
                    sync_info=mybir.SyncInfo(on_wait=[w], on_update=[]),
                )
                super()._commit_instruction(nop, lazy_reg_writes=False)
        return super()._commit_instruction(inst, lazy_reg_writes)

    def _drain_and_barrier(self, tick_clock, wait_clock):
        carrier = self.nc.sync.drain()
        wait_clock.add_sem_waits(
            carrier.ins, ScopedClock({None: tick_clock.global_clock})
        )
        si = carrier.ins.sync_info
        waits = list(si.on_wait) if si is not None else []
        if len(waits) > 1:
            si.on_wait[:] = waits[:1]
            for w in waits[1:]:
                extra = self.nc.sync.drain()
                extra.ins.sync_info = mybir.SyncInfo(on_wait=[w], on_update=[])
        self.nc.all_engine_barrier()
        assert self.sems is not None
        popped = self.nc._tile_sem_poison_stack.pop()
        assert popped is self._sem_poison
        self.nc.clear_and_free_semaphores(list(self.sems.allocated().values()))
        self.nc.all_engine_barrier()


def build_nc():
    nc = bass.Bass("TRN2", target_bir_lowering=False)
    xTb = nc.declare_dram_parameter("xTb", [E, C], BF16, isOutput=False)
    xT8 = nc.declare_dram_parameter("xT8", [E, C], FP8, isOutput=False)
    Wq8 = nc.declare_dram_parameter("Wq8", [E, GF], FP8, isOutput=False)
    Wk8 = nc.declare_dram_parameter("Wk8", [E, GF], FP8, isOutput=False)
    Wv = nc.declare_dram_parameter("Wv", [E, GF], BF16, isOutput=False)
    Wo = nc.declare_dram_parameter("Wo", [GF, E], BF16, isOutput=False)
    # triangular strip mask (k<=q within a 128x128 diagonal sub-block)
    mtri = nc.declare_dram_parameter("mtri", [128, KB], BF16, isOutput=False)
    # wide masks for qc=0 (zeros below the strip + triangle), width 128*(dr+1)
    mwide = nc.declare_dram_parameter("mwide", [128, 4 * QC], BF16, isOutput=False)
    out = nc.declare_dram_parameter("out", [C, E], F32, isOutput=True)

    xTb_t = xTb.ap().rearrange("(po pi) f -> pi po f", pi=128)  # [128, 8, C]
    xT8_t = xT8.ap().rearrange("(po pi) f -> pi po f", pi=128)
    Wq8_t = Wq8.ap().rearrange("(po pi) f -> pi po f", pi=128)  # [128, 8, 512]
    Wk8_t = Wk8.ap().rearrange("(po pi) f -> pi po f", pi=128)
    Wv_t = Wv.ap().rearrange("(po pi) f -> pi po f", pi=128)
    Wo_t = Wo.ap().rearrange("(po pi) f -> pi po f", pi=128)    # [128, 4, E]

    with PatchedTC(nc) as tc:
        import contextlib

        with contextlib.ExitStack() as ctx:
            consts = ctx.enter_context(tc.tile_pool(name="consts", bufs=1))
            ppsum = ctx.enter_context(tc.tile_pool(name="ppsum", bufs=2, space="PSUM"))
            xpool = ctx.enter_context(tc.tile_pool(name="xpool", bufs=1))
            vpool = ctx.enter_context(tc.tile_pool(name="vpool", bufs=1))
            qkpool = ctx.enter_context(tc.tile_pool(name="qkpool", bufs=1))
            stpool = ctx.enter_context(tc.tile_pool(name="stpsum", bufs=2, space="PSUM"))
            hidpool = ctx.enter_context(tc.tile_pool(name="hidpsum", bufs=1, space="PSUM"))
            wtpool = ctx.enter_context(tc.tile_pool(name="wtpool", bufs=3))
            napool = ctx.enter_context(tc.tile_pool(name="napool", bufs=2))
            hfpool = ctx.enter_context(tc.tile_pool(name="hfpool", bufs=1))
            ostage = ctx.enter_context(tc.tile_pool(name="ostage", bufs=2))

            mtri_sb = consts.tile([128, KB], BF16)
            mwide_sb = consts.tile([128, 4, QC], BF16)
            nc.sync.dma_start(mtri_sb[:], mtri.ap())
            nc.sync.dma_start(
                mwide_sb[:], mwide.ap().rearrange("p (r q) -> p r q", q=QC)
            )

            # DMA issue order tuned for startup: wv + first x_bf16 chunks
            # (V projections), then fp8 weights + x_fp8 (QK projections),
            # then the rest of x_bf16, then Wo (needed last).
            wv_sb = consts.tile([128, NE, GF], BF16)
            wq_sb = consts.tile([128, NE, GF], FP8)
            wk_sb = consts.tile([128, NE, GF], FP8)
            wo_sb = consts.tile([128, 4, E], BF16)
            x8_sb = xpool.tile([128, NE, C], FP8)
            xb_sb = xpool.tile([128, NE, C], BF16)
            nc.sync.dma_start(wv_sb[:], Wv_t[:])
            HC = C // 2
            for e in range(NE):
                nc.sync.dma_start(xb_sb[:, e, 0:HC], xTb_t[:, e, 0:HC])
            nc.sync.dma_start(wq_sb[:], Wq8_t[:])
            nc.sync.dma_start(wk_sb[:], Wk8_t[:])
            for e in range(NE):
                nc.sync.dma_start(x8_sb[:, e, 0:HC], xT8_t[:, e, 0:HC])
            for e in range(NE):
                nc.sync.dma_start(xb_sb[:, e, HC:C], xTb_t[:, e, HC:C])
            for e in range(NE):
                nc.sync.dma_start(x8_sb[:, e, HC:C], xT8_t[:, e, HC:C])
            nc.sync.dma_start(wo_sb[:], Wo_t[:])

            # v_sb: per k-block tile, 8 heads x 128 cols.  Even head of a
            # pair: [V(64) | ones(64)]; odd head: [ones | V].
            v_sb = vpool.tile([128, NKB, 2 * GF], BF16)
            nc.any.memset(v_sb[:], 1.0)

            # Q^T/K^T per head-pair: [128, C] f32r (head A rows 0:64,
            # head B rows 64:128).  Plain fp8 score matmuls measurably slow
            # the whole chip ~20% (f32r row-tiled pairs, as in the original
            # kernel, run at full clock), so scores stay f32r.
            qt = []
            kt = []
            for pp in range(4):
                qtp = qkpool.tile([128, C], F32R, tag=f"qt{pp}", name=f"qt{pp}")
                ktp = qkpool.tile([128, C], F32R, tag=f"kt{pp}", name=f"kt{pp}")
                qt.append(qtp)
                kt.append(ktp)

            hf = hfpool.tile([128, 4, C], BF16)  # hidT: 4 head-pairs x 64+64

            def emit_v(t):
                pv = ppsum.tile([128, GF], F32, tag="ppsum")
                for e in range(NE):
                    nc.tensor.matmul(
                        pv[:],
                        lhsT=xb_sb[:, e, t * KB : (t + 1) * KB],
                        rhs=wv_sb[:, e, :],
                        start=(e == 0),
                        stop=(e == NE - 1),
                    )
                src = pv[:].rearrange("p (h u) -> p h u", u=64)
                dst = v_sb[:, t, :].rearrange("p (h u) -> p h u", u=128)
                nc.vector.tensor_copy(dst[:, 0:8:2, 0:64], src[:, 0:8:2, :])
                nc.vector.tensor_copy(dst[:, 1:8:2, 64:128], src[:, 1:8:2, :])

            def emit_qk(hp, which, n):
                # one 512-token fp8-DoubleRow psum chain for head-pair hp
                # (DoubleRow pairs e-tiles: 4 matmuls contract all of E)
                w_sb = wq_sb if which == 0 else wk_sb
                dst = qt[hp] if which == 0 else kt[hp]
                pq = ppsum.tile([128, QC], F32, tag="ppsum")
                for i in range(4):
                    nc.tensor.matmul(
                        pq[:],
                        lhsT=w_sb[:, 2 * i : 2 * i + 2, hp * 128 : (hp + 1) * 128],
                        rhs=x8_sb[:, 2 * i : 2 * i + 2, n * QC : (n + 1) * QC],
                        start=(i == 0),
                        stop=(i == 3),
                        perf_mode=DR,
                    )
                nc.vector.tensor_copy(dst[:, n * QC : (n + 1) * QC], pq[:])

            def emit_o(qb):
                for ec in range(E // QC):
                    po = ppsum.tile([128, QC], F32, tag="ppsum")
                    for f in range(4):
                        nc.tensor.matmul(
                            po[:],
                            lhsT=hf[:, f, qb * KB : (qb + 1) * KB],
                            rhs=wo_sb[:, f, ec * QC : (ec + 1) * QC],
                            start=(f == 0),
                            stop=(f == 3),
                        )
                    so = ostage.tile([128, QC], F32, tag="so")
                    nc.vector.tensor_copy(so[:], po[:])
                    nc.sync.dma_start(
                        out.ap()[qb * KB : (qb + 1) * KB, ec * QC : (ec + 1) * QC],
                        so[:],
                    )

            # ---- filler work queue --------------------------------------
            # entries: (ready_step, deadline_step, emit_fn).  A filler may
            # only be EMITTED at sweep position >= ready (tile-framework deps
            # are tracked in emission order, so e.g. an O-projection emitted
            # before its q-chunk's hidT writes would read unwritten data);
            # it MUST be emitted before position == deadline to avoid stalls.
            sweep = [
                (0, 0), (0, 1), (1, 0), (1, 1),
                (0, 2), (0, 3), (1, 2), (1, 3),
                (2, 0), (2, 1), (2, 2), (2, 3),
                (3, 0), (3, 1), (3, 2), (3, 3),
            ]
            qc_done_after = {0: 5, 1: 7, 2: 11, 3: 15}

            fq = []
            for n in range(NQC):
                for which in range(2):
                    fq.append((0, 1, lambda n=n, w=which: emit_qk(1, w, n)))
            for t in range(4, 8):
                fq.append((0, 2, lambda t=t: emit_v(t)))
            for hp in (2, 3):
                for n in range(NQC):
                    for which in range(2):
                        fq.append(
                            (0, 4, lambda hp=hp, n=n, w=which: emit_qk(hp, w, n))
                        )
            for t in range(8, 12):
                fq.append((0, 8, lambda t=t: emit_v(t)))
            for t in range(12, 16):
                fq.append((0, 12, lambda t=t: emit_v(t)))
            for qcj in range(3):
                for qb in range(4 * qcj, 4 * qcj + 4):
                    fq.append(
                        (qc_done_after[qcj] + 1, 16, lambda qb=qb: emit_o(qb))
                    )

            # ---- phase A: V(0..3) + head-pair 0 QK projections ----------
            for t in range(4):
                emit_v(t)
            for n in range(NQC):
                for which in range(2):
                    emit_qk(0, which, n)

            # ---- phase B: attention sweep with interleaved fillers ------
            def pop_due(step):
                # force-emit everything whose deadline has arrived
                i = 0
                while i < len(fq):
                    ready, deadline, fn = fq[i]
                    if deadline <= step:
                        assert ready <= step
                        fq.pop(i)
                        fn()
                    else:
                        i += 1

            def pop_one(step):
                for i, (ready, deadline, fn) in enumerate(fq):
                    if ready <= step:
                        fq.pop(i)
                        fn()
                        return

            pending_norm = [None]

            def emit_norm(hidA, hidB, hp, qc):
                # 1/rowsum via exp(-ln(rs)) on ACT; hidA rowsum at rows
                # 64:128, hidB ([ones|V]) rowsum at rows 0:64.
                lnA = napool.tile([64, QC], F32, tag="ln")
                recA = napool.tile([64, QC], F32, tag="rec")
                lnB = napool.tile([64, QC], F32, tag="ln")
                recB = napool.tile([64, QC], F32, tag="rec")
                nc.scalar.activation(
                    lnA[:], hidA[64:128, :], mybir.ActivationFunctionType.Ln
                )
                nc.scalar.activation(
                    recA[:], lnA[:], mybir.ActivationFunctionType.Exp, scale=-1.0
                )
                nc.scalar.activation(
                    lnB[:], hidB[0:64, :], mybir.ActivationFunctionType.Ln
                )
                nc.scalar.activation(
                    recB[:], lnB[:], mybir.ActivationFunctionType.Exp, scale=-1.0
                )
                nc.vector.tensor_tensor(
                    hf[0:64, hp, qc * QC : (qc + 1) * QC],
                    hidA[0:64, :],
                    recA[:],
                    mybir.AluOpType.mult,
                )
                nc.vector.tensor_tensor(
                    hf[64:128, hp, qc * QC : (qc + 1) * QC],
                    hidB[64:128, :],
                    recB[:],
                    mybir.AluOpType.mult,
                )

            for step, (qc, hp) in enumerate(sweep):
                # anything whose deadline is this step must be emitted now
                pop_due(step)
                nkb = 4 * qc + 4
                # diag-first order: full-width dr0 opens the accumulation,
                # a full-width block closes it (qc=0 runs untrimmed)
                kbs = [nkb - 4 + dr for dr in range(4)] + list(range(nkb - 4))

                def s_off(kb):
                    dr = kb - (nkb - 4)
                    return 128 * dr if (dr >= 0 and qc > 0) else 0

                def emit_s(kb):
                    off = s_off(kb)
                    st = stpool.tile([128, 2, QC], F32, tag="st")
                    for h in range(2):
                        nc.tensor.matmul(
                            st[:, h, off:QC],
                            lhsT=kt[hp][64 * h : 64 * h + 64, kb * KB : (kb + 1) * KB],
                            rhs=qt[hp][64 * h : 64 * h + 64, qc * QC + off : (qc + 1) * QC],
                            start=True,
                            stop=True,
                        )
                    return st

                hidA = hidpool.tile([128, QC], F32, tag="hidA")
                hidB = hidpool.tile([128, QC], F32, tag="hidB")
                # scores lead the exp->mask->hid chain by one block so the
                # ACT exp stream never waits on the tensor engine
                st = emit_s(kbs[0])
                for j, kb in enumerate(kbs):
                    dr = kb - (nkb - 4)
                    diag = dr >= 0
                    off = s_off(kb)
                    wt = wtpool.tile([128, 2, QC], BF16, tag="wt")
                    eoff = 0 if qc == 0 else off
                    nc.scalar.activation(
                        wt[:, :, eoff:QC],
                        st[:, :, eoff:QC],
                        mybir.ActivationFunctionType.Exp,
                        scale=EXPSCALE,
                    )
                    if diag:
                        if qc == 0:
                            # zeros below the strip + triangle, width 128(dr+1)
                            wdr = 128 * (dr + 1)
                            nc.vector.tensor_tensor(
                                wt[:, :, 0:wdr],
                                wt[:, :, 0:wdr],
                                mwide_sb[:, None, dr, 0:wdr].to_broadcast(
                                    (128, 2, wdr)
                                ),
                                mybir.AluOpType.mult,
                            )
                        else:
                            nc.vector.tensor_tensor(
                                wt[:, :, off : off + KB],
                                wt[:, :, off : off + KB],
                                mtri_sb[:, None, :].to_broadcast((128, 2, KB)),
                                mybir.AluOpType.mult,
                            )
                    if j + 1 < len(kbs):
                        st = emit_s(kbs[j + 1])
                    if j == 0 and pending_norm[0] is not None:
                        # previous chunk's normalize lands here so its four
                        # ACT passes ride behind this chunk's first exp
                        pending_norm[0]()
                        pending_norm[0] = None
                    pop_one(step)
                    first, last = (j == 0), (j == len(kbs) - 1)
                    nc.tensor.matmul(
                        hidA[:, off:QC],
                        lhsT=v_sb[:, kb, (2 * hp) * KB : (2 * hp + 1) * KB],
                        rhs=wt[:, 0, off:QC],
                        start=first,
                        stop=last,
                    )
                    nc.tensor.matmul(
                        hidB[:, off:QC],
                        lhsT=v_sb[:, kb, (2 * hp + 1) * KB : (2 * hp + 2) * KB],
                        rhs=wt[:, 1, off:QC],
                        start=first,
                        stop=last,
                    )
                pending_norm[0] = (
                    lambda a=hidA, b=hidB, hp=hp, qc=qc: emit_norm(a, b, hp, qc)
                )
            pending_norm[0]()

            # drain remaining fillers + final q-chunk's output projection
            while fq:
                fq.pop(0)[2]()
            for qb in range(12, 16):
                emit_o(qb)
    return nc


def _make_masks():
    import ml_dtypes

    kk = np.arange(128)[:, None]
    tri = (kk <= np.arange(KB)[None, :]).astype(np.float32)
    wide = np.zeros((128, 4, QC), dtype=np.float32)
    for dr in range(4):
        qq = np.arange(QC)[None, :]
        wide[:, dr, :] = (128 * dr + kk <= qq).astype(np.float32)
    return (
        tri.astype(ml_dtypes.bfloat16),
        np.ascontiguousarray(wide.reshape(128, 4 * QC)).astype(ml_dtypes.bfloat16),
    )


def make_in_maps(x, W_q, W_k, W_v, W_o):
    import ml_dtypes

    bf16 = ml_dtypes.bfloat16
    e4 = ml_dtypes.float8_e4m3fn
    mtri, mwide = _make_masks()
    in_maps = []
    for i in range(N_CORES):
        b, g = i // 2, i % 2
        xT = np.ascontiguousarray(np.asarray(x)[b].T)
        wq = np.asarray(W_q)[:, g * GF : (g + 1) * GF] * SC
        wk = np.asarray(W_k)[:, g * GF : (g + 1) * GF] * SC
        in_maps.append(
            {
                "xTb": xT.astype(bf16),
                "xT8": xT.astype(e4),
                "Wq8": np.ascontiguousarray(wq).astype(e4),
                "Wk8": np.ascontiguousarray(wk).astype(e4),
                "Wv": np.ascontiguousarray(
                    np.asarray(W_v)[:, g * GF : (g + 1) * GF]
                ).astype(bf16),
                "Wo": np.ascontiguousarray(
                    np.asarray(W_o)[g * GF : (g + 1) * GF, :]
                ).astype(bf16),
                "mtri": mtri,
                "mwide": mwide,
            }
        )
    return in_maps


def kernel(x, W_q, W_k, W_v, W_o):
    global _CACHED_NC
    from concourse.bass_utils import run_bass_kernel_spmd

    if _CACHED_NC is None:
        _CACHED_NC = build_nc()
    nc = _CACHED_NC

    in_maps = make_in_maps(x, W_q, W_k, W_v, W_o)
    res = run_bass_kernel_spmd(nc, in_maps, core_ids=list(range(N_CORES)))
    out = np.empty((B, C, E), dtype=np.float32)
    for b in range(B):
        out[b] = res.results[2 * b]["out"] + res.results[2 * b + 1]["out"]
    return out
